# revision 34
# baseline (speedup 1.0000x reference)
"""Single-head attention (B=4, T=4096, D=1024, H=64) on 8 TRN2 NeuronCores.

Sharding: data-parallel over B (4 batches x 2 cores); within a batch each
core owns 2048 q rows and streams the batch's full kv set.

Device kernel (bf16 compute, f32 softmax accumulation):
  - kv compaction on host: unmasked kv rows first (NKV=2176 = 2048+4sigma),
    filler killed by the exp bias.
  - x arrives HOST-PRE-TRANSPOSED: xqt/xkvt are [128, DC, T*] bf16 so all
    loads are plain contiguous DMAs. xq blocks issue on the scalar HWDGE
    ring, xkv + weights on the sync ring (parallel descriptor generation).
  - q projection uses a [wq|wq] stationary so qT lands duplicated in both
    partition halves; kT is duplicated via a DVE shift-copy. This enables
    QK row-tiling: even s-chunks contract in PE rows 0-63, odd chunks in
    rows 64-127, two K=64 QK matmuls streaming concurrently.
  - ONE flat attention pipeline across both q halves (18 pair-steps): QK
    pair at step j, exp (ACT, mask bias) at j-1, PV accumulate at j-2.
    Late projections are injected INTO the pipeline (PE queue is in-order;
    emitting them up front would gate attention start).  Finalize of half 0
    overlaps half 1's pipeline; only the PSUM->SBUF stage copy gates the
    accumulator reuse.
  - PE warmup matmuls + exp table load at t~0 (identity built before the
    gpsimd const DMAs so nothing gates them).
"""
import numpy as np
import ml_dtypes

import concourse.bass as bass
import concourse.mybir as mybir
from concourse import bacc
from concourse.tile import TileContext
from concourse.masks import make_identity
from concourse.bass_utils import run_bass_kernel_spmd

B, T, D, H = 4, 4096, 1024, 64
N_CORES = 8
TQ = T // 2            # q rows per core
QB = TQ // 512         # q 512-col blocks
DC = D // 128          # contraction chunks
NKV = 2176             # compacted kv positions (binomial 2048+-32, +4 sigma)
SCK = NKV // 128       # kv chunks of 128
SCALE = float(H) ** -0.5

F32 = mybir.dt.float32
BF16 = mybir.dt.bfloat16

NKVP = 9               # kv pair-blocks of 256 cols (last holds 128)
N_WARM = 48            # PE warmup matmuls while first DMA blocks stream


def build_kernel():
    nc = bacc.Bacc()
    # blocked layouts: per partition each DMA block is contiguous
    xqt = nc.dram_tensor("xqt", [128, QB, DC, 512], BF16, kind="ExternalInput")
    xkvt = nc.dram_tensor("xkvt", [128, NKVP, DC, 256], BF16,
                          kind="ExternalInput")  # last block zero-padded
    # per dc: cols 0-127 [wq|wq], 128-191 wk, 192-255 wv
    wt = nc.dram_tensor("wt", [128, DC, 256], BF16, kind="ExternalInput")
    # col0 [bq|bq], col1 [bk|bv] (bv folded into v), cols 2:2+SCK exp mask bias
    cmb = nc.dram_tensor("cmb", [128, 2 + SCK], F32, kind="ExternalInput")
    out = nc.dram_tensor("out", [TQ, H], F32, kind="ExternalOutput")

    with TileContext(nc) as tc:
        with tc.tile_pool(name="const", bufs=1) as const, \
             tc.tile_pool(name="vstage", bufs=2) as vstage, \
             tc.tile_pool(name="ptile", bufs=3) as ptile, \
             tc.tile_pool(name="ostage", bufs=2) as ostage, \
             tc.tile_pool(name="rec", bufs=4) as recp, \
             tc.tile_pool(name="proj", bufs=2, space="PSUM") as projp, \
             tc.tile_pool(name="pqk", bufs=2, space="PSUM") as pqkp, \
             tc.tile_pool(name="pso", bufs=1, space="PSUM") as psop:
            # identity first: nothing on the gpsimd queue ahead of it
            ident32 = const.tile([128, 128], F32)
            make_identity(nc, ident32)
            identb = const.tile([128, 128], BF16)
            nc.vector.tensor_copy(identb, ident32)

            # exp table-set load + PE warmup while DMAs stream
            actwarm = vstage.tile([128, 1], F32)
            nc.scalar.activation(actwarm, ident32[:, 0:1],
                                 mybir.ActivationFunctionType.Exp)
            warm_ps = projp.tile([128, 128], F32, tag="proj", name="warm")
            for _ in range(N_WARM):
                nc.tensor.matmul(warm_ps, identb, identb, start=True, stop=True)

            # consts lead the sync chain (full HBM bandwidth, ~1.5us)
            wt_sb = const.tile([128, DC, 256], BF16)
            nc.sync.dma_start(out=wt_sb, in_=wt.ap()[:, :, :])
            cmb_sb = const.tile([128, 2 + SCK], F32)
            nc.sync.dma_start(out=cmb_sb, in_=cmb.ap()[:, :])
            biasc_sb = cmb_sb[:, 0:2]
            maskb_sb = cmb_sb[:, 2:2 + SCK]

            xqT = const.tile([128, QB, DC, 512], BF16)
            xkvT = const.tile([128, NKVP, DC, 256], BF16)
            qT2 = const.tile([128, TQ], BF16)    # rows 0-63 q, 64-127 q (dup)
            kT2 = const.tile([128, NKV], BF16)   # rows 0-63 k, 64-127 k (dup)
            v_sb = const.tile([128, SCK, H + 1], BF16)
            out_sb = const.tile([128, TQ // 128, H], F32)

            nc.vector.memset(v_sb[:, :, H:H + 1], 1.0)

            # Need-order load chain.  Tile/NRT spread concurrent dma_starts
            # over logical queues which the SDMA engines drain round-robin,
            # so without gating ALL loads finish together (late).  Gate DMA i
            # on completion of DMA i-2 via a tiny gpsimd "touch" that reads a
            # slice of i-2's destination (RAW) and of i's destination (so the
            # DMA write gets a WAR on it): <=2 transfers live at any time.
            touch_sb = vstage.tile([128, 1], BF16, name="touch_sb", bufs=1)
            load_chain = ([("q", 0), ("q", 1)]
                          + [("kv", p) for p in range(6)]
                          + [("q", 2), ("kv", 6), ("kv", 7),
                             ("q", 3), ("kv", 8)])

            def _ldslice(item):
                kind, bi = item
                return xqT[:, bi, 0, 0:1] if kind == "q" else xkvT[:, bi, 0, 0:1]

            for i, item in enumerate(load_chain):
                if i >= 3:
                    nc.gpsimd.tensor_add(
                        touch_sb, _ldslice(load_chain[i - 3]), _ldslice(item))
                kind, bi = item
                if kind == "q":
                    nc.sync.dma_start(out=xqT[:, bi], in_=xqt.ap()[:, bi])
                else:
                    nc.sync.dma_start(out=xkvT[:, bi], in_=xkvt.ap()[:, bi])

            def qproj(tb, hh):
                csl = slice(tb * 512 + hh * 256, tb * 512 + (hh + 1) * 256)
                ps_q = projp.tile([128, 256], F32, tag="proj",
                                  name=f"psq{tb}_{hh}")
                for dc in range(DC):
                    nc.tensor.matmul(
                        ps_q, wt_sb[:, dc, 0:128],
                        xqT[:, tb, dc, hh * 256:(hh + 1) * 256],
                        start=(dc == 0), stop=(dc == DC - 1))
                nc.vector.tensor_scalar_add(
                    qT2[:, csl], ps_q, biasc_sb[:, 0:1])

            def kvproj(p):
                sz = 256 if p < NKVP - 1 else 128
                nch = sz // 128
                ssl = slice(p * 256, p * 256 + sz)
                ps_kv = projp.tile([128, 256], F32, tag="proj", name=f"pskv{p}")
                for dc in range(DC):
                    nc.tensor.matmul(
                        ps_kv[:, 0:sz], wt_sb[:, dc, 128:256],
                        xkvT[:, p, dc, 0:sz],
                        start=(dc == 0), stop=(dc == DC - 1))
                # k with bias into both partition halves (row-tiling operands)
                nc.vector.tensor_scalar_add(
                    kT2[0:64, ssl], ps_kv[0:H, 0:sz], biasc_sb[0:H, 1:2])
                nc.vector.tensor_scalar_add(
                    kT2[64:128, ssl], ps_kv[0:H, 0:sz], biasc_sb[0:H, 1:2])
                # v+bv -> [s, h] via PE transpose (bv rides rows 64-127 of col1)
                vt = vstage.tile([H, 256], BF16, name=f"vt{p}")
                nc.vector.tensor_scalar_add(
                    vt[:, 0:sz], ps_kv[H:128, 0:sz], biasc_sb[64:128, 1:2])
                psvt = projp.tile([128, 2, H], BF16, tag="proj",
                                  name=f"psvt{p}")
                for j in range(nch):
                    nc.tensor.transpose(
                        psvt[:, j, :], vt[:, j * 128:(j + 1) * 128],
                        identb[0:H, 0:H])
                nc.vector.tensor_copy(
                    v_sb[:, 2 * p:2 * p + nch, 0:H], psvt[:, 0:nch, :])

            # ---- attention pipeline machinery ----
            qk_tiles = {}
            p_tiles = {}
            pso_tiles = {}
            o_stash = {}

            def emit_qk(tbp, sc):
                half = slice(0, 64) if sc % 2 == 0 else slice(64, 128)
                ps_qk = pqkp.tile([128, 1024], F32, tag="pqk",
                                  name=f"pqk{sc % 2}")
                for j in range(2):
                    tb = 2 * tbp + j
                    nc.tensor.matmul(
                        ps_qk[:, j * 512:(j + 1) * 512],
                        kT2[half, sc * 128:(sc + 1) * 128],
                        qT2[half, tb * 512:(tb + 1) * 512],
                        start=True, stop=True)
                qk_tiles[(tbp, sc)] = ps_qk

            def emit_exp(tbp, sc):
                p = ptile.tile([128, 1024], BF16, name=f"p{sc % 3}")
                nc.scalar.activation(
                    p, qk_tiles.pop((tbp, sc)),
                    mybir.ActivationFunctionType.Exp,
                    bias=maskb_sb[:, sc:sc + 1], scale=SCALE)
                p_tiles[(tbp, sc)] = p

            def emit_pv(tbp, sc):
                p = p_tiles.pop((tbp, sc))
                for j in range(2):
                    nc.tensor.matmul(
                        pso_tiles[tbp][j], v_sb[:, sc, :],
                        p[:, j * 512:(j + 1) * 512],
                        start=(sc == 0), stop=(sc == SCK - 1))

            def alloc_pso(tbp):
                pso_tiles[tbp] = [
                    psop.tile([H + 1, 512], F32, tag=f"pso{j}",
                              name=f"pso{tbp}_{j}")
                    for j in range(2)
                ]

            def stage_out(tbp):
                for j in range(2):
                    o_sb = ostage.tile([H + 1, 512], F32, name=f"osb{tbp}_{j}")
                    nc.vector.tensor_copy(o_sb, pso_tiles[tbp][j])
                    o_stash[(tbp, j)] = o_sb

            def finalize_rest(tbp, j):
                o_sb = o_stash.pop((tbp, j))
                tb = 2 * tbp + j
                ps_ot = projp.tile([128, 4, H + 1], F32, tag="proj",
                                   name=f"psot{tb}")
                for i in range(4):
                    nc.tensor.transpose(
                        ps_ot[:, i, :], o_sb[:, i * 128:(i + 1) * 128],
                        ident32[0:H + 1, 0:H + 1])
                rec = recp.tile([128, 4, 1], F32, name=f"rec{tb}")
                nc.vector.reciprocal(rec, ps_ot[:, :, H:H + 1])
                for i in range(4):
                    nc.vector.tensor_scalar_mul(
                        out_sb[:, 4 * tb + i, :], ps_ot[:, i, 0:H],
                        rec[:, i, :])

            out_r = out.rearrange("(i p) h -> p i h", p=128)

            def emit_out_dma(tbp):
                nc.sync.dma_start(
                    out=out_r[:, 8 * tbp:8 * (tbp + 1), :],
                    in_=out_sb[:, 8 * tbp:8 * (tbp + 1), :])

            # pair-steps: [(tbp, sc), ...] per step; 9 per tbp (last is lone)
            steps = []
            for tbp in range(2):
                sc = 0
                while sc < SCK:
                    n = 2 if sc + 1 < SCK else 1
                    steps.append([(tbp, s) for s in range(sc, sc + n)])
                    sc += n
            nsteps = len(steps)

            # work injected into the pipeline (PE queue is in-order);
            # group at j precedes the QK of step j+1 (emitted after QK(j))
            inject = {
                0: [("kv", 1)],
                1: [("kv", 2)],
                2: [("kv", 3)],
                3: [("kv", 4)],
                4: [("kv", 5), ("q", (2, 0))],
                5: [("kv", 6), ("q", (2, 1))],
                6: [("kv", 7), ("q", (3, 0))],
                7: [("kv", 8), ("q", (3, 1))],
            }

            qproj(0, 0)
            qproj(0, 1)
            qproj(1, 0)
            qproj(1, 1)
            kvproj(0)
            alloc_pso(0)

            for j in range(nsteps + 2):
                if 1 <= j < nsteps + 1:
                    for tbp, sc in steps[j - 1]:
                        emit_exp(tbp, sc)
                # QK first on the PE queue: it gates the next exp; PV and
                # injected projections fill the remaining step slack
                if j < nsteps:
                    for tbp, sc in steps[j]:
                        emit_qk(tbp, sc)
                if j >= 2:
                    for tbp, sc in steps[j - 2]:
                        emit_pv(tbp, sc)
                    if steps[j - 2][0][1] == SCK - 1:  # last PV of this tbp
                        stage_out(steps[j - 2][0][0])
                        if steps[j - 2][0][0] == 0:
                            alloc_pso(1)
                if j >= 2 and j - 2 < len(steps) and steps[j - 2][0][0] == 1 \
                        and steps[j - 2][0][1] == 2:
                    # half 0 accumulators staged two steps ago: finalize now
                    finalize_rest(0, 0)
                    finalize_rest(0, 1)
                    emit_out_dma(0)
                for kind, arg in inject.get(j, []):
                    if kind == "kv":
                        kvproj(arg)
                    else:
                        qproj(*arg)

            finalize_rest(1, 0)
            nc.sync.dma_start(out=out_r[:, 8:12, :], in_=out_sb[:, 8:12, :])
            finalize_rest(1, 1)
            nc.sync.dma_start(out=out_r[:, 12:16, :], in_=out_sb[:, 12:16, :])

    nc.finalize()
    return nc


_NC_CACHE = None


def _get_nc():
    global _NC_CACHE
    if _NC_CACHE is None:
        _NC_CACHE = build_kernel()
    return _NC_CACHE


def make_in_maps(x, mask, wq, bq, wk, bk, wv, bv):
    x = np.asarray(x, dtype=np.float32)
    mask = np.asarray(mask)
    wq = np.asarray(wq, np.float32)
    wk = np.asarray(wk, np.float32)
    wv = np.asarray(wv, np.float32)

    # stationary weights [128, DC, 256]: per dc [wq|wq|wk|wv]
    wqr = wq.T.reshape(DC, 128, H)
    wkr = wk.T.reshape(DC, 128, H)
    wvr = wv.T.reshape(DC, 128, H)
    wt = np.concatenate([wqr, wqr, wkr, wvr], axis=2)  # [DC, 128, 256]
    wt = np.ascontiguousarray(wt.transpose(1, 0, 2)).astype(ml_dtypes.bfloat16)

    bqf = np.asarray(bq, np.float32)
    bkf = np.asarray(bk, np.float32)
    bvf = np.asarray(bv, np.float32)
    biasc = np.stack([np.concatenate([bqf, bqf]),
                      np.concatenate([bkf, bvf])], axis=1)

    in_maps = []
    per_batch = {}
    for b in range(B):
        mb = mask[b].astype(bool)
        keep = np.flatnonzero(mb)
        fill = np.flatnonzero(~mb)
        cnt = len(keep)
        assert cnt <= NKV, f"unmasked kv count {cnt} exceeds NKV={NKV}"
        order = np.concatenate([keep, fill])[:NKV]
        xkv = x[b][order]  # [NKV, D]
        # blocked [128, NKVP, DC, 256], last block zero-padded to 256
        xkvt = np.zeros((128, NKVP, DC, 256), dtype=ml_dtypes.bfloat16)
        xkvb = xkv.T.reshape(DC, 128, NKV)  # [dc, p, s]
        for p in range(NKVP):
            sz = min(256, NKV - p * 256)
            blk = xkvb[:, :, p * 256:p * 256 + sz]
            xkvt[:, p, :, 0:sz] = blk.transpose(1, 0, 2)
        biasvals = np.where(np.arange(NKV) < cnt, 0.0, -1e9).astype(np.float32)
        maskb = biasvals.reshape(SCK, 128).T
        cmb = np.ascontiguousarray(
            np.concatenate([biasc, maskb], axis=1)).copy()
        per_batch[b] = (xkvt, cmb)

    for c in range(N_CORES):
        b, half = c // 2, c % 2
        xkvt, cmb = per_batch[b]
        xq = x[b, half * TQ:(half + 1) * TQ]  # [TQ, D]
        xqt = np.ascontiguousarray(
            xq.T.reshape(DC, 128, QB, 512).transpose(1, 2, 0, 3)
        ).astype(ml_dtypes.bfloat16)
        in_maps.append({
            "xqt": xqt,
            "xkvt": xkvt,
            "wt": wt,
            "cmb": cmb,
        })
    return in_maps


def run(in_maps, **kwargs):
    nc = _get_nc()
    return run_bass_kernel_spmd(nc, in_maps, core_ids=list(range(N_CORES)), **kwargs)


def kernel(x, mask, wq, bq, wk, bk, wv, bv):
    in_maps = make_in_maps(x, mask, wq, bq, wk, bk, wv, bv)
    res = run(in_maps)
    out = np.empty((B, T, H), dtype=np.float32)
    for c in range(N_CORES):
        b, half = c // 2, c % 2
        out[b, half * TQ:(half + 1) * TQ] = res.results[c]["out"]
    return out


# revision 36
# speedup vs baseline: 1.1822x; 1.1822x over previous
"""Single-head attention (B=4, T=4096, D=1024, H=64) on 8 TRN2 NeuronCores.

Sharding: data-parallel over B (4 batches x 2 cores); within a batch each
core owns 2048 q rows and streams the batch's full kv set.

Device kernel (bf16 compute, f32 softmax accumulation):
  - kv compaction on host: unmasked kv rows first (NKV=2176 = 2048+4sigma),
    filler killed by the exp bias.
  - x arrives HOST-PRE-TRANSPOSED: xqt/xkvt are [128, DC, T*] bf16 so all
    loads are plain contiguous DMAs. xq blocks issue on the scalar HWDGE
    ring, xkv + weights on the sync ring (parallel descriptor generation).
  - q projection uses a [wq|wq] stationary so qT lands duplicated in both
    partition halves; kT is duplicated via a DVE shift-copy. This enables
    QK row-tiling: even s-chunks contract in PE rows 0-63, odd chunks in
    rows 64-127, two K=64 QK matmuls streaming concurrently.
  - ONE flat attention pipeline across both q halves (18 pair-steps): QK
    pair at step j, exp (ACT, mask bias) at j-1, PV accumulate at j-2.
    Late projections are injected INTO the pipeline (PE queue is in-order;
    emitting them up front would gate attention start).  Finalize of half 0
    overlaps half 1's pipeline; only the PSUM->SBUF stage copy gates the
    accumulator reuse.
  - PE warmup matmuls + exp table load at t~0 (identity built before the
    gpsimd const DMAs so nothing gates them).
"""
import numpy as np
import ml_dtypes

import concourse.bass as bass
import concourse.mybir as mybir
from concourse import bacc
from concourse.tile import TileContext
from concourse.masks import make_identity
from concourse.bass_utils import run_bass_kernel_spmd

B, T, D, H = 4, 4096, 1024, 64
N_CORES = 8
TQ = T // 2            # q rows per core
QB = TQ // 512         # q 512-col blocks
DC = D // 128          # contraction chunks
NKV = 2176             # compacted kv positions (binomial 2048+-32, +4 sigma)
SCK = NKV // 128       # kv chunks of 128
SCALE = float(H) ** -0.5

F32 = mybir.dt.float32
BF16 = mybir.dt.bfloat16

NKVP = 9               # kv pair-blocks of 256 cols (last holds 128)
N_WARM = 48            # PE warmup matmuls while first DMA blocks stream


def build_kernel():
    nc = bacc.Bacc()
    # blocked layouts: per partition each DMA block is contiguous
    xqt = nc.dram_tensor("xqt", [128, QB, DC, 512], BF16, kind="ExternalInput")
    xkvt = nc.dram_tensor("xkvt", [128, NKVP, DC, 256], BF16,
                          kind="ExternalInput")  # last block zero-padded
    # per dc: cols 0-127 [wq|wq], 128-191 wk, 192-255 wv
    wt = nc.dram_tensor("wt", [128, DC, 256], BF16, kind="ExternalInput")
    # col0 [bq|bq], col1 [bk|bv] (bv folded into v), cols 2:2+SCK exp mask bias
    cmb = nc.dram_tensor("cmb", [128, 2 + SCK], F32, kind="ExternalInput")
    out = nc.dram_tensor("out", [TQ, H], F32, kind="ExternalOutput")

    with TileContext(nc) as tc:
        with tc.tile_pool(name="const", bufs=1) as const, \
             tc.tile_pool(name="vstage", bufs=2) as vstage, \
             tc.tile_pool(name="ptile", bufs=5) as ptile, \
             tc.tile_pool(name="ostage", bufs=2) as ostage, \
             tc.tile_pool(name="rec", bufs=4) as recp, \
             tc.tile_pool(name="proj", bufs=2, space="PSUM") as projp, \
             tc.tile_pool(name="pqk", bufs=2, space="PSUM") as pqkp, \
             tc.tile_pool(name="pso", bufs=1, space="PSUM") as psop:
            # identity first: nothing on the gpsimd queue ahead of it
            ident32 = const.tile([128, 128], F32)
            make_identity(nc, ident32)
            identb = const.tile([128, 128], BF16)
            nc.vector.tensor_copy(identb, ident32)

            # exp table-set load + PE warmup while DMAs stream
            actwarm = vstage.tile([128, 1], F32)
            nc.scalar.activation(actwarm, ident32[:, 0:1],
                                 mybir.ActivationFunctionType.Exp)
            warm_ps = projp.tile([128, 128], F32, tag="proj", name="warm")
            for _ in range(N_WARM):
                nc.tensor.matmul(warm_ps, identb, identb, start=True, stop=True)

            # consts lead the sync chain (full HBM bandwidth, ~1.5us)
            wt_sb = const.tile([128, DC, 256], BF16)
            nc.sync.dma_start(out=wt_sb, in_=wt.ap()[:, :, :])
            cmb_sb = const.tile([128, 2 + SCK], F32)
            nc.sync.dma_start(out=cmb_sb, in_=cmb.ap()[:, :])
            biasc_sb = cmb_sb[:, 0:2]
            maskb_sb = cmb_sb[:, 2:2 + SCK]

            xqT = const.tile([128, QB, DC, 512], BF16)
            xkvT = const.tile([128, NKVP, DC, 256], BF16)
            qT2 = const.tile([128, TQ], BF16)    # rows 0-63 q, 64-127 q (dup)
            kT2 = const.tile([128, NKV], BF16)   # rows 0-63 k, 64-127 k (dup)
            v_sb = const.tile([128, SCK, H + 1], BF16)
            out_sb = const.tile([128, TQ // 128, H], F32)

            nc.vector.memset(v_sb[:, :, H:H + 1], 1.0)

            # Need-order load chain.  Tile/NRT spread concurrent dma_starts
            # over logical queues which the SDMA engines drain round-robin,
            # so without gating ALL loads finish together (late).  Gate DMA i
            # on completion of DMA i-2 via a tiny gpsimd "touch" that reads a
            # slice of i-2's destination (RAW) and of i's destination (so the
            # DMA write gets a WAR on it): <=2 transfers live at any time.
            touch_sb = vstage.tile([128, 1], BF16, name="touch_sb", bufs=1)
            load_chain = ([("q", 0), ("q", 1)]
                          + [("kv", p) for p in range(6)]
                          + [("q", 2), ("kv", 6), ("kv", 7),
                             ("q", 3), ("kv", 8)])

            def _ldslice(item):
                kind, bi = item
                return xqT[:, bi, 0, 0:1] if kind == "q" else xkvT[:, bi, 0, 0:1]

            for i, item in enumerate(load_chain):
                if i >= 3:
                    nc.gpsimd.tensor_add(
                        touch_sb, _ldslice(load_chain[i - 3]), _ldslice(item))
                kind, bi = item
                if kind == "q":
                    nc.sync.dma_start(out=xqT[:, bi], in_=xqt.ap()[:, bi])
                else:
                    nc.sync.dma_start(out=xkvT[:, bi], in_=xkvt.ap()[:, bi])

            def qproj(tb, hh):
                csl = slice(tb * 512 + hh * 256, tb * 512 + (hh + 1) * 256)
                ps_q = projp.tile([128, 256], F32, tag="proj",
                                  name=f"psq{tb}_{hh}")
                for dc in range(DC):
                    nc.tensor.matmul(
                        ps_q, wt_sb[:, dc, 0:128],
                        xqT[:, tb, dc, hh * 256:(hh + 1) * 256],
                        start=(dc == 0), stop=(dc == DC - 1))
                nc.vector.tensor_scalar_add(
                    qT2[:, csl], ps_q, biasc_sb[:, 0:1])

            def kvproj(p):
                sz = 256 if p < NKVP - 1 else 128
                nch = sz // 128
                ssl = slice(p * 256, p * 256 + sz)
                ps_kv = projp.tile([128, 256], F32, tag="proj", name=f"pskv{p}")
                for dc in range(DC):
                    nc.tensor.matmul(
                        ps_kv[:, 0:sz], wt_sb[:, dc, 128:256],
                        xkvT[:, p, dc, 0:sz],
                        start=(dc == 0), stop=(dc == DC - 1))
                # k with bias into both partition halves (row-tiling operands)
                nc.vector.tensor_scalar_add(
                    kT2[0:64, ssl], ps_kv[0:H, 0:sz], biasc_sb[0:H, 1:2])
                nc.vector.tensor_scalar_add(
                    kT2[64:128, ssl], ps_kv[0:H, 0:sz], biasc_sb[0:H, 1:2])
                # v+bv -> [s, h] via PE transpose (bv rides rows 64-127 of col1)
                vt = vstage.tile([H, 256], BF16, name=f"vt{p}")
                nc.vector.tensor_scalar_add(
                    vt[:, 0:sz], ps_kv[H:128, 0:sz], biasc_sb[64:128, 1:2])
                psvt = projp.tile([128, 2, H], BF16, tag="proj",
                                  name=f"psvt{p}")
                for j in range(nch):
                    nc.tensor.transpose(
                        psvt[:, j, :], vt[:, j * 128:(j + 1) * 128],
                        identb[0:H, 0:H])
                nc.vector.tensor_copy(
                    v_sb[:, 2 * p:2 * p + nch, 0:H], psvt[:, 0:nch, :])

            # ---- attention pipeline machinery ----
            qk_tiles = {}
            p_tiles = {}
            pso_tiles = {}
            o_stash = {}

            def emit_qk(tbp, sc):
                half = slice(0, 64) if sc % 2 == 0 else slice(64, 128)
                ps_qk = pqkp.tile([128, 1024], F32, tag="pqk",
                                  name=f"pqk{sc % 2}")
                for j in range(2):
                    tb = 2 * tbp + j
                    nc.tensor.matmul(
                        ps_qk[:, j * 512:(j + 1) * 512],
                        kT2[half, sc * 128:(sc + 1) * 128],
                        qT2[half, tb * 512:(tb + 1) * 512],
                        start=True, stop=True)
                qk_tiles[(tbp, sc)] = ps_qk

            def emit_exp(tbp, sc):
                p = ptile.tile([128, 1024], BF16, name=f"p{sc % 5}")
                nc.scalar.activation(
                    p, qk_tiles.pop((tbp, sc)),
                    mybir.ActivationFunctionType.Exp,
                    bias=maskb_sb[:, sc:sc + 1], scale=SCALE)
                p_tiles[(tbp, sc)] = p

            def emit_pv(tbp, sc):
                p = p_tiles.pop((tbp, sc))
                for j in range(2):
                    nc.tensor.matmul(
                        pso_tiles[tbp][j], v_sb[:, sc, :],
                        p[:, j * 512:(j + 1) * 512],
                        start=(sc == 0), stop=(sc == SCK - 1))

            def alloc_pso(tbp):
                pso_tiles[tbp] = [
                    psop.tile([H + 1, 512], F32, tag=f"pso{j}",
                              name=f"pso{tbp}_{j}")
                    for j in range(2)
                ]

            def stage_out(tbp):
                for j in range(2):
                    o_sb = ostage.tile([H + 1, 512], F32, name=f"osb{tbp}_{j}")
                    nc.vector.tensor_copy(o_sb, pso_tiles[tbp][j])
                    o_stash[(tbp, j)] = o_sb

            def finalize_rest(tbp, j):
                o_sb = o_stash.pop((tbp, j))
                tb = 2 * tbp + j
                ps_ot = projp.tile([128, 4, H + 1], F32, tag="proj",
                                   name=f"psot{tb}")
                for i in range(4):
                    nc.tensor.transpose(
                        ps_ot[:, i, :], o_sb[:, i * 128:(i + 1) * 128],
                        ident32[0:H + 1, 0:H + 1])
                rec = recp.tile([128, 4, 1], F32, name=f"rec{tb}")
                nc.vector.reciprocal(rec, ps_ot[:, :, H:H + 1])
                for i in range(4):
                    nc.vector.tensor_scalar_mul(
                        out_sb[:, 4 * tb + i, :], ps_ot[:, i, 0:H],
                        rec[:, i, :])

            out_r = out.rearrange("(i p) h -> p i h", p=128)

            def emit_out_dma(tbp):
                nc.sync.dma_start(
                    out=out_r[:, 8 * tbp:8 * (tbp + 1), :],
                    in_=out_sb[:, 8 * tbp:8 * (tbp + 1), :])

            # pair-steps: [(tbp, sc), ...] per step; 9 per tbp (last is lone)
            steps = []
            for tbp in range(2):
                sc = 0
                while sc < SCK:
                    n = 2 if sc + 1 < SCK else 1
                    steps.append([(tbp, s) for s in range(sc, sc + n)])
                    sc += n
            nsteps = len(steps)

            # work injected into the pipeline (PE queue is in-order);
            # group at j precedes the QK of step j+1 (emitted after QK(j))
            inject = {
                0: [("kv", 1)],
                1: [("kv", 2)],
                2: [("kv", 3)],
                3: [("kv", 4)],
                4: [("kv", 5), ("q", (2, 0))],
                5: [("kv", 6), ("q", (2, 1))],
                6: [("kv", 7), ("q", (3, 0))],
                7: [("kv", 8), ("q", (3, 1))],
            }

            qproj(0, 0)
            qproj(0, 1)
            qproj(1, 0)
            qproj(1, 1)
            kvproj(0)
            alloc_pso(0)

            for j in range(nsteps + 2):
                if 1 <= j < nsteps + 1:
                    for tbp, sc in steps[j - 1]:
                        emit_exp(tbp, sc)
                # QK first on the PE queue: it gates the next exp; PV and
                # injected projections fill the remaining step slack
                if j < nsteps:
                    for tbp, sc in steps[j]:
                        emit_qk(tbp, sc)
                if j >= 2:
                    for tbp, sc in steps[j - 2]:
                        emit_pv(tbp, sc)
                    if steps[j - 2][0][1] == SCK - 1:  # last PV of this tbp
                        stage_out(steps[j - 2][0][0])
                        if steps[j - 2][0][0] == 0:
                            alloc_pso(1)
                if j >= 2 and j - 2 < len(steps) and steps[j - 2][0][0] == 1 \
                        and steps[j - 2][0][1] == 2:
                    # half 0 accumulators staged two steps ago: finalize now
                    finalize_rest(0, 0)
                    finalize_rest(0, 1)
                    emit_out_dma(0)
                for kind, arg in inject.get(j, []):
                    if kind == "kv":
                        kvproj(arg)
                    else:
                        qproj(*arg)

            finalize_rest(1, 0)
            nc.sync.dma_start(out=out_r[:, 8:12, :], in_=out_sb[:, 8:12, :])
            finalize_rest(1, 1)
            nc.sync.dma_start(out=out_r[:, 12:16, :], in_=out_sb[:, 12:16, :])

    nc.finalize()
    return nc


_NC_CACHE = None


def _get_nc():
    global _NC_CACHE
    if _NC_CACHE is None:
        _NC_CACHE = build_kernel()
    return _NC_CACHE


def make_in_maps(x, mask, wq, bq, wk, bk, wv, bv):
    x = np.asarray(x, dtype=np.float32)
    mask = np.asarray(mask)
    wq = np.asarray(wq, np.float32)
    wk = np.asarray(wk, np.float32)
    wv = np.asarray(wv, np.float32)

    # stationary weights [128, DC, 256]: per dc [wq|wq|wk|wv]
    wqr = wq.T.reshape(DC, 128, H)
    wkr = wk.T.reshape(DC, 128, H)
    wvr = wv.T.reshape(DC, 128, H)
    wt = np.concatenate([wqr, wqr, wkr, wvr], axis=2)  # [DC, 128, 256]
    wt = np.ascontiguousarray(wt.transpose(1, 0, 2)).astype(ml_dtypes.bfloat16)

    bqf = np.asarray(bq, np.float32)
    bkf = np.asarray(bk, np.float32)
    bvf = np.asarray(bv, np.float32)
    biasc = np.stack([np.concatenate([bqf, bqf]),
                      np.concatenate([bkf, bvf])], axis=1)

    in_maps = []
    per_batch = {}
    for b in range(B):
        mb = mask[b].astype(bool)
        keep = np.flatnonzero(mb)
        fill = np.flatnonzero(~mb)
        cnt = len(keep)
        assert cnt <= NKV, f"unmasked kv count {cnt} exceeds NKV={NKV}"
        order = np.concatenate([keep, fill])[:NKV]
        xkv = x[b][order]  # [NKV, D]
        # blocked [128, NKVP, DC, 256], last block zero-padded to 256
        xkvt = np.zeros((128, NKVP, DC, 256), dtype=ml_dtypes.bfloat16)
        xkvb = xkv.T.reshape(DC, 128, NKV)  # [dc, p, s]
        for p in range(NKVP):
            sz = min(256, NKV - p * 256)
            blk = xkvb[:, :, p * 256:p * 256 + sz]
            xkvt[:, p, :, 0:sz] = blk.transpose(1, 0, 2)
        biasvals = np.where(np.arange(NKV) < cnt, 0.0, -1e9).astype(np.float32)
        maskb = biasvals.reshape(SCK, 128).T
        cmb = np.ascontiguousarray(
            np.concatenate([biasc, maskb], axis=1)).copy()
        per_batch[b] = (xkvt, cmb)

    for c in range(N_CORES):
        b, half = c // 2, c % 2
        xkvt, cmb = per_batch[b]
        xq = x[b, half * TQ:(half + 1) * TQ]  # [TQ, D]
        xqt = np.ascontiguousarray(
            xq.T.reshape(DC, 128, QB, 512).transpose(1, 2, 0, 3)
        ).astype(ml_dtypes.bfloat16)
        in_maps.append({
            "xqt": xqt,
            "xkvt": xkvt,
            "wt": wt,
            "cmb": cmb,
        })
    return in_maps


def run(in_maps, **kwargs):
    nc = _get_nc()
    return run_bass_kernel_spmd(nc, in_maps, core_ids=list(range(N_CORES)), **kwargs)


def kernel(x, mask, wq, bq, wk, bk, wv, bv):
    in_maps = make_in_maps(x, mask, wq, bq, wk, bk, wv, bv)
    res = run(in_maps)
    out = np.empty((B, T, H), dtype=np.float32)
    for c in range(N_CORES):
        b, half = c // 2, c % 2
        out[b, half * TQ:(half + 1) * TQ] = res.results[c]["out"]
    return out


# revision 42
# speedup vs baseline: 1.1886x; 1.0054x over previous
"""Single-head attention (B=4, T=4096, D=1024, H=64) on 8 TRN2 NeuronCores.

Sharding: data-parallel over B (4 batches x 2 cores); within a batch each
core owns 2048 q rows and streams the batch's full kv set.

Device kernel (bf16 compute, f32 softmax accumulation):
  - kv compaction on host: unmasked kv rows first (NKV=2176 = 2048+4sigma),
    filler killed by the exp bias.
  - x arrives HOST-PRE-TRANSPOSED: xqt/xkvt are [128, DC, T*] bf16 so all
    loads are plain contiguous DMAs. xq blocks issue on the scalar HWDGE
    ring, xkv + weights on the sync ring (parallel descriptor generation).
  - q projection uses a [wq|wq] stationary so qT lands duplicated in both
    partition halves; kT is duplicated via a DVE shift-copy. This enables
    QK row-tiling: even s-chunks contract in PE rows 0-63, odd chunks in
    rows 64-127, two K=64 QK matmuls streaming concurrently.
  - ONE flat attention pipeline across both q halves (18 pair-steps): QK
    pair at step j, exp (ACT, mask bias) at j-1, PV accumulate at j-2.
    Late projections are injected INTO the pipeline (PE queue is in-order;
    emitting them up front would gate attention start).  Finalize of half 0
    overlaps half 1's pipeline; only the PSUM->SBUF stage copy gates the
    accumulator reuse.
  - PE warmup matmuls + exp table load at t~0 (identity built before the
    gpsimd const DMAs so nothing gates them).
"""
import numpy as np
import ml_dtypes

import concourse.bass as bass
import concourse.mybir as mybir
from concourse import bacc
from concourse.tile import TileContext
from concourse.masks import make_identity
from concourse.bass_utils import run_bass_kernel_spmd

B, T, D, H = 4, 4096, 1024, 64
N_CORES = 8
TQ = T // 2            # q rows per core
QB = TQ // 512         # q 512-col blocks
DC = D // 128          # contraction chunks
NKV = 2176             # compacted kv positions (binomial 2048+-32, +4 sigma)
SCK = NKV // 128       # kv chunks of 128
SCALE = float(H) ** -0.5

F32 = mybir.dt.float32
BF16 = mybir.dt.bfloat16

NKVP = 9               # kv pair-blocks of 256 cols (last holds 128)
N_WARM = 32            # PE warmup matmuls while first DMA blocks stream


def build_kernel():
    nc = bacc.Bacc()
    # blocked layouts: per partition each DMA block is contiguous
    xqt = nc.dram_tensor("xqt", [128, QB, DC, 512], BF16, kind="ExternalInput")
    xkvt = nc.dram_tensor("xkvt", [128, NKVP, DC, 256], BF16,
                          kind="ExternalInput")  # last block zero-padded
    # per dc: cols 0-127 [wq|wq], 128-191 wk, 192-255 wv
    wt = nc.dram_tensor("wt", [128, DC, 256], BF16, kind="ExternalInput")
    # col0 [bq|bq], col1 [bk|bv] (bv folded into v), cols 2:2+SCK exp mask bias
    cmb = nc.dram_tensor("cmb", [128, 2 + SCK], F32, kind="ExternalInput")
    out = nc.dram_tensor("out", [TQ, H], F32, kind="ExternalOutput")

    with TileContext(nc) as tc:
        with tc.tile_pool(name="const", bufs=1) as const, \
             tc.tile_pool(name="vstage", bufs=2) as vstage, \
             tc.tile_pool(name="ptile", bufs=5) as ptile, \
             tc.tile_pool(name="ostage", bufs=2) as ostage, \
             tc.tile_pool(name="rec", bufs=4) as recp, \
             tc.tile_pool(name="proj", bufs=2, space="PSUM") as projp, \
             tc.tile_pool(name="pqk", bufs=2, space="PSUM") as pqkp, \
             tc.tile_pool(name="pso", bufs=1, space="PSUM") as psop:
            # identity first: nothing on the gpsimd queue ahead of it
            ident32 = const.tile([128, 128], F32)
            make_identity(nc, ident32)
            identb = const.tile([128, 128], BF16)
            nc.vector.tensor_copy(identb, ident32)

            # exp table-set load + PE warmup while DMAs stream
            actwarm = vstage.tile([128, 1], F32)
            nc.scalar.activation(actwarm, ident32[:, 0:1],
                                 mybir.ActivationFunctionType.Exp)
            warm_ps = projp.tile([128, 128], F32, tag="proj", name="warm")
            for _ in range(N_WARM):
                nc.tensor.matmul(warm_ps, identb, identb, start=True, stop=True)

            wt_sb = const.tile([128, DC, 256], BF16)
            cmb_sb = const.tile([128, 2 + SCK], F32)
            biasc_sb = cmb_sb[:, 0:2]
            maskb_sb = cmb_sb[:, 2:2 + SCK]

            xqT = const.tile([128, QB, DC, 512], BF16)
            xkvT = const.tile([128, NKVP, DC, 256], BF16)
            qT2 = const.tile([128, TQ], BF16)    # rows 0-63 q, 64-127 q (dup)
            kT2 = const.tile([128, NKV], BF16)   # rows 0-63 k, 64-127 k (dup)
            v_sb = const.tile([128, SCK, H + 1], BF16)
            out_sb = const.tile([128, TQ // 128, H], F32)

            nc.vector.memset(v_sb[:, :, H:H + 1], 1.0)

            # Need-order load chain.  Tile/NRT spread concurrent dma_starts
            # over logical queues which the SDMA engines drain round-robin,
            # so without gating ALL loads finish together (late).  Gate DMA i
            # on completion of DMA i-2 via a tiny gpsimd "touch" that reads a
            # slice of i-2's destination (RAW) and of i's destination (so the
            # DMA write gets a WAR on it): <=2 transfers live at any time.
            touch_sb = vstage.tile([128, 2], BF16, name="touch_sb", bufs=1)
            load_chain = ([("c", 0), ("w", 0), ("q", 0), ("q", 1)]
                          + [("kv", p) for p in range(6)]
                          + [("q", 2), ("kv", 6), ("kv", 7),
                             ("q", 3), ("kv", 8)])

            def _ldslice(item):
                kind, bi = item
                if kind == "q":
                    return xqT[:, bi, 0, 0:2]
                if kind == "kv":
                    return xkvT[:, bi, 0, 0:2]
                if kind == "w":
                    return wt_sb[:, 0, 0:2]
                return cmb_sb[:, 0:1].bitcast(BF16)

            for i, item in enumerate(load_chain):
                if i >= 3:
                    nc.gpsimd.tensor_add(
                        touch_sb, _ldslice(load_chain[i - 3]),
                        _ldslice(item))
                kind, bi = item
                if kind == "q":
                    nc.sync.dma_start(out=xqT[:, bi], in_=xqt.ap()[:, bi])
                elif kind == "kv":
                    nc.sync.dma_start(out=xkvT[:, bi], in_=xkvt.ap()[:, bi])
                elif kind == "w":
                    nc.sync.dma_start(out=wt_sb, in_=wt.ap()[:, :, :])
                else:
                    nc.sync.dma_start(out=cmb_sb, in_=cmb.ap()[:, :])

            def qproj(tb, hh):
                csl = slice(tb * 512 + hh * 256, tb * 512 + (hh + 1) * 256)
                ps_q = projp.tile([128, 256], F32, tag="proj",
                                  name=f"psq{tb}_{hh}")
                for dc in range(DC):
                    nc.tensor.matmul(
                        ps_q, wt_sb[:, dc, 0:128],
                        xqT[:, tb, dc, hh * 256:(hh + 1) * 256],
                        start=(dc == 0), stop=(dc == DC - 1))
                nc.vector.tensor_scalar_add(
                    qT2[:, csl], ps_q, biasc_sb[:, 0:1])

            def kvproj(p):
                sz = 256 if p < NKVP - 1 else 128
                nch = sz // 128
                ssl = slice(p * 256, p * 256 + sz)
                ps_kv = projp.tile([128, 256], F32, tag="proj", name=f"pskv{p}")
                for dc in range(DC):
                    nc.tensor.matmul(
                        ps_kv[:, 0:sz], wt_sb[:, dc, 128:256],
                        xkvT[:, p, dc, 0:sz],
                        start=(dc == 0), stop=(dc == DC - 1))
                # k with bias into both partition halves (row-tiling operands)
                nc.vector.tensor_scalar_add(
                    kT2[0:64, ssl], ps_kv[0:H, 0:sz], biasc_sb[0:H, 1:2])
                nc.vector.tensor_scalar_add(
                    kT2[64:128, ssl], ps_kv[0:H, 0:sz], biasc_sb[0:H, 1:2])
                # v+bv -> [s, h] via PE transpose (bv rides rows 64-127 of col1)
                vt = vstage.tile([H, 256], BF16, name=f"vt{p}")
                nc.vector.tensor_scalar_add(
                    vt[:, 0:sz], ps_kv[H:128, 0:sz], biasc_sb[64:128, 1:2])
                psvt = projp.tile([128, 2, H], BF16, tag="proj",
                                  name=f"psvt{p}")
                for j in range(nch):
                    nc.tensor.transpose(
                        psvt[:, j, :], vt[:, j * 128:(j + 1) * 128],
                        identb[0:H, 0:H])
                nc.vector.tensor_copy(
                    v_sb[:, 2 * p:2 * p + nch, 0:H], psvt[:, 0:nch, :])

            # ---- attention pipeline machinery ----
            qk_tiles = {}
            p_tiles = {}
            pso_tiles = {}
            o_stash = {}

            def emit_qk(tbp, sc):
                half = slice(0, 64) if sc % 2 == 0 else slice(64, 128)
                ps_qk = pqkp.tile([128, 1024], F32, tag="pqk",
                                  name=f"pqk{sc % 2}")
                for j in range(2):
                    tb = 2 * tbp + j
                    nc.tensor.matmul(
                        ps_qk[:, j * 512:(j + 1) * 512],
                        kT2[half, sc * 128:(sc + 1) * 128],
                        qT2[half, tb * 512:(tb + 1) * 512],
                        start=True, stop=True)
                qk_tiles[(tbp, sc)] = ps_qk

            def emit_exp(tbp, sc):
                p = ptile.tile([128, 1024], BF16, name=f"p{sc % 5}")
                nc.scalar.activation(
                    p, qk_tiles.pop((tbp, sc)),
                    mybir.ActivationFunctionType.Exp,
                    bias=maskb_sb[:, sc:sc + 1], scale=SCALE)
                p_tiles[(tbp, sc)] = p

            def emit_pv(tbp, sc):
                p = p_tiles.pop((tbp, sc))
                for j in range(2):
                    nc.tensor.matmul(
                        pso_tiles[tbp][j], v_sb[:, sc, :],
                        p[:, j * 512:(j + 1) * 512],
                        start=(sc == 0), stop=(sc == SCK - 1))

            def alloc_pso(tbp):
                pso_tiles[tbp] = [
                    psop.tile([H + 1, 512], F32, tag=f"pso{j}",
                              name=f"pso{tbp}_{j}")
                    for j in range(2)
                ]

            def stage_out(tbp):
                for j in range(2):
                    o_sb = ostage.tile([H + 1, 512], F32, name=f"osb{tbp}_{j}")
                    if j == 0:
                        nc.vector.tensor_copy(o_sb, pso_tiles[tbp][j])
                    else:
                        nc.scalar.copy(o_sb, pso_tiles[tbp][j])
                    o_stash[(tbp, j)] = o_sb

            def finalize_rest(tbp, j):
                o_sb = o_stash.pop((tbp, j))
                tb = 2 * tbp + j
                ps_ot = projp.tile([128, 4, H + 1], F32, tag="proj",
                                   name=f"psot{tb}")
                for i in range(4):
                    nc.tensor.transpose(
                        ps_ot[:, i, :], o_sb[:, i * 128:(i + 1) * 128],
                        ident32[0:H + 1, 0:H + 1])
                rec = recp.tile([128, 4, 1], F32, name=f"rec{tb}")
                nc.vector.reciprocal(rec, ps_ot[:, :, H:H + 1])
                for i in range(4):
                    nc.vector.tensor_scalar_mul(
                        out_sb[:, 4 * tb + i, :], ps_ot[:, i, 0:H],
                        rec[:, i, :])

            out_r = out.rearrange("(i p) h -> p i h", p=128)

            def emit_out_dma(tbp):
                nc.sync.dma_start(
                    out=out_r[:, 8 * tbp:8 * (tbp + 1), :],
                    in_=out_sb[:, 8 * tbp:8 * (tbp + 1), :])

            # pair-steps: [(tbp, sc), ...] per step; 9 per tbp (last is lone)
            steps = []
            for tbp in range(2):
                sc = 0
                while sc < SCK:
                    n = 2 if sc + 1 < SCK else 1
                    steps.append([(tbp, s) for s in range(sc, sc + n)])
                    sc += n
            nsteps = len(steps)

            # work injected into the pipeline (PE queue is in-order);
            # group at j precedes the QK of step j+1 (emitted after QK(j))
            inject = {
                0: [("kv", 1)],
                1: [("kv", 2)],
                2: [("kv", 3)],
                3: [("kv", 4)],
                4: [("kv", 5), ("q", (2, 0))],
                5: [("kv", 6), ("q", (2, 1))],
                6: [("kv", 7), ("q", (3, 0))],
                7: [("kv", 8), ("q", (3, 1))],
            }

            qproj(0, 0)
            qproj(0, 1)
            qproj(1, 0)
            qproj(1, 1)
            kvproj(0)
            alloc_pso(0)

            for j in range(nsteps + 2):
                if 1 <= j < nsteps + 1:
                    for tbp, sc in steps[j - 1]:
                        emit_exp(tbp, sc)
                # QK first on the PE queue: it gates the next exp; PV and
                # injected projections fill the remaining step slack
                if j < nsteps:
                    for tbp, sc in steps[j]:
                        emit_qk(tbp, sc)
                if j >= 2:
                    for tbp, sc in steps[j - 2]:
                        emit_pv(tbp, sc)
                    if steps[j - 2][0][1] == SCK - 1:  # last PV of this tbp
                        stage_out(steps[j - 2][0][0])
                        if steps[j - 2][0][0] == 0:
                            alloc_pso(1)
                if j >= 2 and j - 2 < len(steps) and steps[j - 2][0][0] == 1 \
                        and steps[j - 2][0][1] == 2:
                    # half 0 accumulators staged two steps ago: finalize now
                    finalize_rest(0, 0)
                    finalize_rest(0, 1)
                    emit_out_dma(0)
                for kind, arg in inject.get(j, []):
                    if kind == "kv":
                        kvproj(arg)
                    else:
                        qproj(*arg)

            finalize_rest(1, 0)
            nc.sync.dma_start(out=out_r[:, 8:12, :], in_=out_sb[:, 8:12, :])
            finalize_rest(1, 1)
            nc.sync.dma_start(out=out_r[:, 12:16, :], in_=out_sb[:, 12:16, :])

    nc.finalize()
    return nc


_NC_CACHE = None


def _get_nc():
    global _NC_CACHE
    if _NC_CACHE is None:
        _NC_CACHE = build_kernel()
    return _NC_CACHE


def make_in_maps(x, mask, wq, bq, wk, bk, wv, bv):
    x = np.asarray(x, dtype=np.float32)
    mask = np.asarray(mask)
    wq = np.asarray(wq, np.float32)
    wk = np.asarray(wk, np.float32)
    wv = np.asarray(wv, np.float32)

    # stationary weights [128, DC, 256]: per dc [wq|wq|wk|wv]
    wqr = wq.T.reshape(DC, 128, H)
    wkr = wk.T.reshape(DC, 128, H)
    wvr = wv.T.reshape(DC, 128, H)
    wt = np.concatenate([wqr, wqr, wkr, wvr], axis=2)  # [DC, 128, 256]
    wt = np.ascontiguousarray(wt.transpose(1, 0, 2)).astype(ml_dtypes.bfloat16)

    bqf = np.asarray(bq, np.float32)
    bkf = np.asarray(bk, np.float32)
    bvf = np.asarray(bv, np.float32)
    biasc = np.stack([np.concatenate([bqf, bqf]),
                      np.concatenate([bkf, bvf])], axis=1)

    in_maps = []
    per_batch = {}
    for b in range(B):
        mb = mask[b].astype(bool)
        keep = np.flatnonzero(mb)
        fill = np.flatnonzero(~mb)
        cnt = len(keep)
        assert cnt <= NKV, f"unmasked kv count {cnt} exceeds NKV={NKV}"
        order = np.concatenate([keep, fill])[:NKV]
        xkv = x[b][order]  # [NKV, D]
        # blocked [128, NKVP, DC, 256], last block zero-padded to 256
        xkvt = np.zeros((128, NKVP, DC, 256), dtype=ml_dtypes.bfloat16)
        xkvb = xkv.T.reshape(DC, 128, NKV)  # [dc, p, s]
        for p in range(NKVP):
            sz = min(256, NKV - p * 256)
            blk = xkvb[:, :, p * 256:p * 256 + sz]
            xkvt[:, p, :, 0:sz] = blk.transpose(1, 0, 2)
        biasvals = np.where(np.arange(NKV) < cnt, 0.0, -1e9).astype(np.float32)
        maskb = biasvals.reshape(SCK, 128).T
        cmb = np.ascontiguousarray(
            np.concatenate([biasc, maskb], axis=1)).copy()
        per_batch[b] = (xkvt, cmb)

    for c in range(N_CORES):
        b, half = c // 2, c % 2
        xkvt, cmb = per_batch[b]
        xq = x[b, half * TQ:(half + 1) * TQ]  # [TQ, D]
        xqt = np.ascontiguousarray(
            xq.T.reshape(DC, 128, QB, 512).transpose(1, 2, 0, 3)
        ).astype(ml_dtypes.bfloat16)
        in_maps.append({
            "xqt": xqt,
            "xkvt": xkvt,
            "wt": wt,
            "cmb": cmb,
        })
    return in_maps


def run(in_maps, **kwargs):
    nc = _get_nc()
    return run_bass_kernel_spmd(nc, in_maps, core_ids=list(range(N_CORES)), **kwargs)


def kernel(x, mask, wq, bq, wk, bk, wv, bv):
    in_maps = make_in_maps(x, mask, wq, bq, wk, bk, wv, bv)
    res = run(in_maps)
    out = np.empty((B, T, H), dtype=np.float32)
    for c in range(N_CORES):
        b, half = c // 2, c % 2
        out[b, half * TQ:(half + 1) * TQ] = res.results[c]["out"]
    return out
